# Initial kernel scaffold
#
"""Trainium2 Bass kernel for nn_EquivariantCorrectionHead.

Strategy: pure data-parallel over 8 NeuronCores (batch 131072 -> 16384/core).
Per core, feature-major layout [features on partitions, batch on free dim],
tiles of NB=512 items. All linear maps / broadcasts / contractions run on the
TensorEngine against host-precomputed constant matrices; the per-item bilinear
products run on the VectorEngine. The CG tensor C222 is CP-decomposed exactly
(symmetric rank 13 for the t x t -> 2e path, with 5 extra basis directions
folding the l=0 Gram path into the same product family; non-symmetric rank 10
for the b2 path), which cuts the bilinear product count ~3x vs the reference
formulation.
"""
import numpy as np

# ---------------------------------------------------------------------------
# constants of the problem (hardcoded per harness contract)
# ---------------------------------------------------------------------------
B_FULL = 131072
N_CORES = 8
B_CORE = B_FULL // N_CORES
NB = 512
S, H, NL2, NK = 16, 32, 9, 40
INV_SQRT5 = float(1.0 / np.sqrt(5.0))
L2_IDX = np.array([0, 1, 2, 4, 24, 26, 35, 38])
PAIRS = [(u, v) for u in range(9) for v in range(u, 9)]   # 45 sym pairs
NDIR = 18                                                  # 13 CP + 5 basis
NP_P, NP_SS, NP_ST = NDIR * 45, 256, 720

# exact CP factors of the 2e x 2e -> 2e CG tensor (see module docstring)
A2 = np.array([[-0.00880792389997489, 0.0255090096975797 , 0.0103778757480062 ,-0.05626541244740764,-0.01112912828217646, 0.01732247542992058, 0.03410740042311852,-0.03216337844207943,-0.00625850211629469, 0.02265767980944357],
 [ 0.02154881168452435, 0.01807304106800752,-0.0184113923823477 ,-0.04260584152443667,-0.01501924024446535,-0.08603477648376368,-0.01579012192635746,-0.04119232769877183, 0.01781007256758009,-0.05413529473857265],
 [ 0.02341490377893025, 0.04563678014869373, 0.03285159604771626,-0.0525188379402777 , 0.02740626807571844,-0.02123616135069552,-0.0066858166891036 , 0.00400491528630738,-0.02059123345090396, 0.00634462454889838],
 [-0.03145722067562591,-0.0223041735669847 ,-0.00271821028037091, 0.11117091976335136,-0.01250885508154663, 0.00484295703373329, 0.03833473157514697,-0.03558034978181717, 0.00459682755285227,-0.02706055497126852],
 [ 0.01091977978077357,-0.06135640098989507,-0.03325620820957877, 0.0296833173858063 , 0.00595693090641491,-0.05707709297095041, 0.01576767514676052, 0.0159498234083972 , 0.00160114911006148,-0.00297734299672801]])
B2 = np.array([[ 0.5415530557436292 ,-1.024908341393839  ,-1.0223202798777546 , 0.2260729898788277 , 4.898835138192793  ,-0.7154915309341058 ,-0.10985634074550359,-2.5194419752235104 , 2.9042259287050527 ,-0.6103486976519019 ],
 [-1.4764672489242259 , 3.911848427368901  , 1.7267096101189925 , 1.462896625832539  ,-1.9982941000780714 ,-0.9660640162932947 ,-1.2572279425167532 , 2.068774160086907  ,-1.6777691108132833 ,-0.3434246927381564 ],
 [-2.1843758378126665 ,-0.11666744824202176, 0.7828859160378078 , 0.2345184082802281 ,-2.6799972851062868 ,-2.070384075779163  , 1.1455382664805225 ,-1.4707055161830553 ,-4.558779029428765  ,-1.8201771207145185 ],
 [ 2.828647951973164  , 0.5419806790638542 , 1.0207126704482592 ,-1.1166083158561817 , 0.4303229535806376 , 1.1496984579803795 ,-2.002369320793801  , 0.3751600762680648 ,-1.863183302411589  ,-0.6424607470143069 ],
 [-0.9524844452334826 ,-2.3078406977616446 ,-2.5539853629582963 ,-0.4452758746877629 ,-0.8463005819465791 ,-2.3740542465423067 ,-0.42752112416823096, 0.20145348882631411, 1.3413701137422653 ,-0.5442104256920791 ]])
C2 = np.array([[ 0.6392765696054369 ,-0.4693363475443954 , 1.3817203703348497 , 0.2775711165956856 ,-2.384005760434029  ,-0.3534688361385708 ,-0.16227860449614406,-1.6156207517079955 ,-1.617176839410101  , 1.769431878310822  ],
 [-1.689148478640906  , 2.0649010313735836 ,-2.767142487527258  , 1.63510107321956   , 1.1048218248281616 ,-0.4792117345500623 ,-1.2952898416347285 , 1.4638341059612259 , 1.3148960367472247 , 0.5719383195517783 ],
 [-2.439251912963143  , 0.28300884960428596,-2.097451215169065  , 0.45545141726388655, 1.8422229767248532 ,-0.8737023695357936 , 0.7590880368180523 ,-0.5668235208487564 , 4.153041443469627  , 3.3169431625711425 ],
 [ 3.3128306513923227 , 0.45030913341800965,-1.995432760784938  ,-1.1155791706004317 , 0.03543421280946218, 0.7740304394864133 ,-2.1282581747263767 , 0.4603345289491318 , 1.8256727487469075 , 0.6040798977591221 ],
 [-1.0826345576730565 ,-1.1039229132376611 , 3.6151916895321636 ,-0.442615899393151  , 0.5311342885572051 ,-1.2553932185713805 ,-0.49181302586044023, 0.22280738628415303,-0.5631916648337107 , 1.3042567455452807 ]])
ASYM = np.array([[ 0.2047078304993985 ,-0.02548683359407013, 0.7272382102103669 ,-0.2704580317002371 , 0.09837678436495051, 0.33917102586453507, 0.0702064199526067 , 0.5084911526521594 ,-0.45926938484350616, 0.02051018350271685, 0.42935279562152645, 0.11369761887680929,-0.9795087183109351 ],
 [-0.44463451059315895,-0.1475020911181585 ,-0.08599458327748657,-0.3399741021461676 ,-0.22682371559002337, 0.28678061126448023, 0.7650776592713625 , 0.26958836857825846, 0.5278386781630274 , 0.3282383438246536 , 0.08456455835271014, 0.5900296552329473 , 0.02292460782275062],
 [-0.4191472648275923 ,-0.26988537144017594,-0.4746358369743323 ,-0.05420760101850775,-0.4844170977223217 ,-0.2282388774655017 ,-0.19634218768794168, 0.08251395533362854, 0.06550659513246503, 0.1133290752849004 , 0.303297071331556  ,-0.5334125260375588 ,-0.16883005035035203],
 [ 1.0431848707368094 ,-0.14228996865607693, 0.1431376570259985 , 0.8838003679813345 ,-0.11670899310031788, 0.21559606010496696, 0.04691847768104187, 0.7674518688427294 , 1.1874537614603238 ,-0.10792487711796182,-0.17088928262877545, 0.09873177011237796,-0.6460911914396512 ],
 [-0.1789979061960668 , 0.6605325263316313 ,-0.04352029718970135, 0.19469438466538228, 0.12156843143529865, 0.5275314988902706 ,-0.7974376738648722 , 0.01570195752313255,-0.1619243884486304 , 0.467054091034758  , 0.16454467309626772,-0.11642783633169705, 0.01683699581923372]])
LAM = np.array([-0.3368296096552994 , 1.2424482608763587 , 0.6885666883749189 , 0.5830944196804277 ,-2.0867522613313056 , 0.21906413438838154, 0.4065311860292724 , 0.7616168984284204 ,-0.16794491943022935,-1.819621132649064  ,-0.6515708567347953 ,-0.6783119354005673 , 0.34147667194459136])

_NC_CACHE = {}


def _build_constant_arrays(w000, w110, w011, w101, w111, v010, v100, v110):
    """Host precompute of every device-resident constant matrix (float32)."""
    c0 = (1.0 / (S * S + 81)) ** 0.5
    c2 = (5.0 / (18 * S + 81)) ** 0.5
    d = (5.0 / (3 * H * H)) ** 0.5
    R1 = 13
    dirs = np.concatenate([ASYM.T, np.eye(5)], axis=0)     # [18, 5]

    C = {}
    Msel = np.zeros((200, 45))
    for v in range(9):
        for j in range(5):
            if v < 8:
                Msel[5 * L2_IDX[v] + j, 5 * v + j] = 1.0
            else:
                for n in range(NK):
                    Msel[5 * n + j, 5 * v + j] = 1.0
    C["Msel0"], C["Msel1"] = Msel[:128], Msel[128:]

    AU = np.zeros((45, NP_P)); AV = np.zeros((45, NP_P))
    for r in range(NDIR):
        for p, (u, v) in enumerate(PAIRS):
            for i in range(5):
                AU[5 * u + i, 45 * r + p] += dirs[r, i]
                AV[5 * v + i, 45 * r + p] += dirs[r, i]
    C["AU"], C["AV"] = AU, AV

    W_P = np.zeros((NP_P, 192))
    wp111 = np.zeros((45, 32)); wp110 = np.zeros((45, 32))
    for p, (u, v) in enumerate(PAIRS):
        if u == v:
            wp111[p], wp110[p] = w111[u, u, :], w110[u, u, :]
        else:
            wp111[p] = w111[u, v, :] + w111[v, u, :]
            wp110[p] = w110[u, v, :] + w110[v, u, :]
    for r in range(R1):
        for k in range(5):
            W_P[45 * r:45 * (r + 1), 32 + 32 * k:64 + 32 * k] = (
                c2 * LAM[r] * ASYM[k, r]) * wp111
    for i in range(5):
        W_P[45 * (R1 + i):45 * (R1 + i + 1), 0:32] = (c0 * INV_SQRT5) * wp110
    W_P = np.concatenate([W_P[:, 32:192], W_P[:, 0:32]], axis=1)
    for c in range(7):
        C[f"WP{c}"] = W_P[128 * c:128 * (c + 1)]

    SSA = np.zeros((16, 256))
    for u in range(16):
        SSA[u, 16 * u:16 * (u + 1)] = 1.0
    C["SSA"] = SSA
    SAmap = np.zeros((16, 128))
    for p in range(128):
        SAmap[p % 16, p] = 1.0
    C["SAmap"] = SAmap
    W_SS = np.zeros((256, 64))
    for u in range(16):
        for v in range(16):
            W_SS[16 * u + v, 32:64] = c0 * w000[u, v, :]
    C["WSS0"], C["WSS1"] = W_SS[:128], W_SS[128:]

    TB = np.zeros((45, NP_ST))
    W_ST = np.zeros((NP_ST, 192))
    for k in range(5):
        for v in range(9):
            for u in range(16):
                q = 144 * k + 16 * v + u
                TB[5 * v + k, q] = 1.0
                W_ST[q, 32 + 32 * k:64 + 32 * k] += c2 * INV_SQRT5 * (
                    w011[u, v, :] + w101[v, u, :])
    W_ST = np.concatenate([W_ST[:, 32:192], W_ST[:, 0:32]], axis=1)
    for c in range(6):
        C[f"TB{c}"] = TB[:, 128 * c:min(NP_ST, 128 * (c + 1))]
        C[f"WST{c}"] = W_ST[128 * c:min(NP_ST, 128 * (c + 1))]

    R2 = 10
    HRm = np.zeros((160, 32 * R2)); ARm = np.zeros((160, 32 * R2))
    for r in range(R2):
        for w in range(32):
            for k in range(5):
                HRm[32 * k + w, 32 * r + w] = A2[k, r]
            for j in range(5):
                for v in range(32):
                    ARm[32 * j + v, 32 * r + w] = B2[j, r] * v110[w, v]
    C["HRa"], C["HRb"] = HRm[:128], HRm[128:]
    C["ARa"], C["ARb"] = ARm[:128], ARm[128:]
    W_B2 = np.zeros((32 * R2, 5))
    for r in range(R2):
        for w in range(32):
            W_B2[32 * r + w] = d * C2[:, r]
    C["WB20"], C["WB21"], C["WB22"] = W_B2[:128], W_B2[128:256], W_B2[256:]

    EB = np.zeros((32, 160))
    for u in range(32):
        for k in range(5):
            for w in range(32):
                EB[u, 32 * k + w] = v010[u, w] + v100[w, u]
    C["EB"] = EB
    W_V = np.zeros((160, 5))
    for k in range(5):
        for w in range(32):
            W_V[32 * k + w, k] = d * INV_SQRT5
    C["WVa"], C["WVb"] = W_V[:128], W_V[128:]

    return {k: np.ascontiguousarray(v, dtype=np.float32) for k, v in C.items()}


CONST_SHAPES = {
    "Msel0": (128, 45), "Msel1": (72, 45),
    "AU": (45, 810), "AV": (45, 810),
    **{f"WP{c}": (min(810, 128 * (c + 1)) - 128 * c, 192) for c in range(7)},
    "SSA": (16, 256), "SAmap": (16, 128),
    "WSS0": (128, 64), "WSS1": (128, 64),
    **{f"TB{c}": (45, min(720, 128 * (c + 1)) - 128 * c) for c in range(6)},
    **{f"WST{c}": (min(720, 128 * (c + 1)) - 128 * c, 192) for c in range(6)},
    "HRa": (128, 320), "HRb": (32, 320), "ARa": (128, 320), "ARb": (32, 320),
    "WB20": (128, 5), "WB21": (128, 5), "WB22": (64, 5),
    "EB": (32, 160), "WVa": (128, 5), "WVb": (32, 5),
}


def build_nc(b_core=B_CORE, repeat=1):
    import concourse.bacc as bacc
    import concourse.mybir as mybir
    import concourse.tile as tile

    f32 = mybir.dt.float32
    nt = b_core // NB
    nc = bacc.Bacc()

    s_dram = nc.dram_tensor("s_t", (16, b_core), f32, kind="ExternalInput")
    kt_dram = nc.dram_tensor("kt_t", (200, b_core), f32, kind="ExternalInput")
    cdram = {k: nc.dram_tensor(k, shp, f32, kind="ExternalInput")
             for k, shp in CONST_SHAPES.items()}
    out_dram = nc.dram_tensor("out_t", (5, b_core), f32, kind="ExternalOutput")

    with tile.TileContext(nc) as tc:
        with (
            tc.tile_pool(name="consts", bufs=1) as cp,
            tc.tile_pool(name="io", bufs=3) as io,
            tc.tile_pool(name="work", bufs=2) as wk,
            tc.tile_pool(name="psum", bufs=1, space="PSUM") as ps,
        ):
            ct = {}
            for k, shp in CONST_SHAPES.items():
                ct[k] = cp.tile(list(shp), f32, tag=k, name=f"c_{k}")
                nc.sync.dma_start(ct[k][:], cdram[k][:])

            for it in range(nt * repeat):
                c0 = NB * (it % nt)
                sl = slice(c0, c0 + NB)

                kt0 = io.tile([128, NB], f32, tag="kt0")
                kt1 = io.tile([72, NB], f32, tag="kt1")
                sT = io.tile([16, NB], f32, tag="sT")
                nc.sync.dma_start(kt0[:], kt_dram[0:128, sl])
                nc.sync.dma_start(kt1[:], kt_dram[128:200, sl])
                nc.sync.dma_start(sT[:], s_dram[:, sl])

                # t45 = Msel.T @ kt
                t45_ps = ps.tile([45, NB], f32, tag="sm", bufs=2)
                nc.tensor.matmul(t45_ps[:], ct["Msel0"][:], kt0[:], start=True, stop=False)
                nc.tensor.matmul(t45_ps[:], ct["Msel1"][:], kt1[:], start=False, stop=True)
                t45 = wk.tile([45, NB], f32, tag="t45")
                nc.vector.tensor_copy(t45[:], t45_ps[:])

                # SA shared pattern s[p % 16]
                SA_ps = ps.tile([128, NB], f32, tag="sm", bufs=2)
                nc.tensor.matmul(SA_ps[:], ct["SAmap"][:], sT[:], start=True, stop=True)
                SA = wk.tile([128, NB], f32, tag="SA")
                nc.scalar.copy(SA[:], SA_ps[:])

                o1a = ps.tile([128, NB], f32, tag="o1a", bufs=2)
                o1b = ps.tile([64, NB], f32, tag="o1b", bufs=2)

                # ---- P family: products ta_r[u] * ta_r[v] over 45 sym pairs x 18 dirs
                for c in range(7):
                    lo, hi = 128 * c, min(NP_P, 128 * (c + 1))
                    n = hi - lo
                    au = ps.tile([n, NB], f32, tag="plc", bufs=1)
                    av = ps.tile([n, NB], f32, tag="plc2", bufs=1)
                    nc.tensor.matmul(au[:], ct["AU"][:, lo:hi], t45[:], start=True, stop=True)
                    nc.tensor.matmul(av[:], ct["AV"][:, lo:hi], t45[:], start=True, stop=True)
                    avs = wk.tile([n, NB], f32, tag="avs", bufs=3)
                    if c % 2 == 0:
                        nc.scalar.copy(avs[:], av[:])
                    else:
                        nc.vector.tensor_copy(avs[:], av[:])
                    pp = wk.tile([n, NB], f32, tag="pp", bufs=3)
                    nc.vector.tensor_mul(pp[:], au[:], avs[:])
                    nc.tensor.matmul(o1a[:], ct[f"WP{c}"][:n, 0:128], pp[:],
                                     start=(c == 0), stop=False)
                    nc.tensor.matmul(o1b[:], ct[f"WP{c}"][:n, 128:192], pp[:],
                                     start=(c == 0), stop=False)


                # ---- SS family: s_u * s_v
                for c in range(2):
                    ssa = ps.tile([128, NB], f32, tag="plc", bufs=1)
                    nc.tensor.matmul(ssa[:], ct["SSA"][:, 128 * c:128 * (c + 1)], sT[:],
                                     start=True, stop=True)
                    pss = wk.tile([128, NB], f32, tag="pss", bufs=3)
                    nc.vector.tensor_mul(pss[:], ssa[:], SA[:])
                    nc.tensor.matmul(o1b[:], ct[f"WSS{c}"][:], pss[:],
                                     start=False, stop=False)

                # ---- ST family: s_u * t45[v,k], q = 144k + 16v + u
                for c in range(6):
                    lo, hi = 128 * c, min(NP_ST, 128 * (c + 1))
                    n = hi - lo
                    tb = ps.tile([n, NB], f32, tag="plc2", bufs=1)
                    nc.tensor.matmul(tb[:], ct[f"TB{c}"][:], t45[:], start=True, stop=True)
                    pst = wk.tile([n, NB], f32, tag="pst", bufs=3)
                    nc.vector.tensor_mul(pst[:], tb[:], SA[:n, :])
                    last = (c == 5)
                    nc.tensor.matmul(o1a[:], ct[f"WST{c}"][:, 0:128], pst[:],
                                     start=False, stop=last)
                    nc.tensor.matmul(o1b[:], ct[f"WST{c}"][:, 128:192], pst[:],
                                     start=False, stop=last)

                # OUT1 -> SBUF: h2 rows 0..127, h2 rows 128..159, h0 [32]
                o1s0 = wk.tile([128, NB], f32, tag="o1s0")
                h24s = wk.tile([32, NB], f32, tag="h24s")
                h0s = wk.tile([32, NB], f32, tag="h0s")
                nc.scalar.copy(o1s0[:], o1a[:])
                nc.vector.tensor_copy(h24s[:], o1b[0:32, :])
                nc.vector.tensor_copy(h0s[:], o1b[32:64, :])

                final_ps = ps.tile([5, NB], f32, tag="sm", bufs=2)

                # ---- v010/v100 path: E-broadcast * h2
                eb_a = ps.tile([128, NB], f32, tag="o1a", bufs=2)
                nc.tensor.matmul(eb_a[:], ct["EB"][:, 0:128], h0s[:], start=True, stop=True)
                pv_a = wk.tile([128, NB], f32, tag="pva", bufs=3)
                nc.vector.tensor_mul(pv_a[:], eb_a[:], o1s0[:])
                nc.tensor.matmul(final_ps[:], ct["WVa"][:], pv_a[:], start=True, stop=False)
                eb_b = ps.tile([32, NB], f32, tag="o1b", bufs=2)
                nc.tensor.matmul(eb_b[:], ct["EB"][:, 128:160], h0s[:], start=True, stop=True)
                pv_b = wk.tile([32, NB], f32, tag="pvb", bufs=3)
                nc.vector.tensor_mul(pv_b[:], eb_b[:], h24s[:])
                nc.tensor.matmul(final_ps[:], ct["WVb"][:], pv_b[:], start=False, stop=False)

                # ---- b2 path via nonsym CP (R=10): HR .* AR, 3 M-pieces
                for mc, (lo, hi) in enumerate(((0, 128), (128, 256), (256, 320))):
                    n = hi - lo
                    hr = ps.tile([n, NB], f32, tag="plc", bufs=1)
                    ar = ps.tile([n, NB], f32, tag="plc2", bufs=1)
                    nc.tensor.matmul(hr[:], ct["HRa"][:, lo:hi], o1s0[:],
                                     start=True, stop=False)
                    nc.tensor.matmul(hr[:], ct["HRb"][:, lo:hi], h24s[:],
                                     start=False, stop=True)
                    nc.tensor.matmul(ar[:], ct["ARa"][:, lo:hi], o1s0[:],
                                     start=True, stop=False)
                    nc.tensor.matmul(ar[:], ct["ARb"][:, lo:hi], h24s[:],
                                     start=False, stop=True)
                    hrs = wk.tile([n, NB], f32, tag="hrs", bufs=3)
                    if mc % 2 == 0:
                        nc.vector.tensor_copy(hrs[:], hr[:])
                    else:
                        nc.scalar.copy(hrs[:], hr[:])
                    pb = wk.tile([n, NB], f32, tag="pb", bufs=3)
                    nc.vector.tensor_mul(pb[:], ar[:], hrs[:])
                    nc.tensor.matmul(final_ps[:], ct[f"WB2{mc}"][:], pb[:],
                                     start=False, stop=(mc == 2))

                out_s = wk.tile([5, NB], f32, tag="outs")
                nc.vector.tensor_copy(out_s[:], final_ps[:])
                nc.sync.dma_start(out_dram[:, sl], out_s[:])

    nc.compile()
    return nc


def _host_prep(scalars, kernel_t2s):
    s_t = np.ascontiguousarray(scalars.T.astype(np.float32, copy=False))
    kt_t = np.ascontiguousarray(
        kernel_t2s.reshape(B_FULL, 200).T.astype(np.float32, copy=False))
    return s_t, kt_t


def kernel(scalars, kernel_t2s, w000, w110, w011, w101, w111, v010, v100, v110):
    from concourse.bass_utils import run_bass_kernel_spmd

    consts = _build_constant_arrays(
        np.asarray(w000, np.float64), np.asarray(w110, np.float64),
        np.asarray(w011, np.float64), np.asarray(w101, np.float64),
        np.asarray(w111, np.float64), np.asarray(v010, np.float64),
        np.asarray(v100, np.float64), np.asarray(v110, np.float64))
    s_t, kt_t = _host_prep(np.asarray(scalars), np.asarray(kernel_t2s))

    if "nc" not in _NC_CACHE:
        _NC_CACHE["nc"] = build_nc()
    nc = _NC_CACHE["nc"]

    in_maps = []
    for c in range(N_CORES):
        sl = slice(c * B_CORE, (c + 1) * B_CORE)
        m = {"s_t": np.ascontiguousarray(s_t[:, sl]),
             "kt_t": np.ascontiguousarray(kt_t[:, sl])}
        m.update(consts)
        in_maps.append(m)

    res = run_bass_kernel_spmd(nc, in_maps, core_ids=list(range(N_CORES)))
    out = np.empty((B_FULL, 5), np.float32)
    for c in range(N_CORES):
        out[c * B_CORE:(c + 1) * B_CORE] = res.results[c]["out_t"].T
    return out



# revision 2
# speedup vs baseline: 1.1518x; 1.1518x over previous
"""Trainium2 Bass kernel for nn_EquivariantCorrectionHead — v2.

Data-parallel over 8 cores (batch 131072 -> 16384/core), feature-major layout,
NB=512 item tiles. All per-item bilinear work is restructured into
(a) squares of linear forms (TensorE prep matmul -> ActE square -> TensorE
    consume matmul) via an exact pair-square basis, and
(b) a small products family for the s x t bilinear path (TensorE prep ->
    VectorE multiply -> TensorE consume).
The CG tensor C222 enters via a symmetric CP decomposition (rank R); the
l=0 Gram path reuses the same square family via a tight-frame expansion of
I5 over the CP directions (or explicit identity directions as fallback).
Everything runs in bf16 on the PE (1 cycle/row vs 4 for fp32).
"""
import numpy as np

import os
MM_DT = os.environ.get("K2_MM_DT", "f32r")   # "bf16" | "f32r" (prep-path f32r hybrid)
PREP_KEYS = ("PR", "TB", "SAW", "FW", "G4")              # consts whose rhs comes straight from DMA


def _is_prep(k):
    return any(k.startswith(pk) for pk in PREP_KEYS)


def _tf32_round(x):
    u = np.ascontiguousarray(x, np.float32).view(np.uint32)
    add = np.uint32(0x0FFF) + ((u >> np.uint32(13)) & np.uint32(1))
    return ((u + add) & np.uint32(0xFFFFE000)).view(np.float32)
try:
    from ml_dtypes import bfloat16 as _bf16np
except Exception:  # pragma: no cover
    import jax.numpy as _jnp
    _bf16np = _jnp.bfloat16

# ---------------------------------------------------------------------------
# problem constants
# ---------------------------------------------------------------------------
B_FULL = 131072
N_CORES = 8
B_CORE = B_FULL // N_CORES
NB = 512
S, H, NL2, NK = 16, 32, 9, 40
INV_SQRT5 = float(1.0 / np.sqrt(5.0))
L2_IDX = np.array([0, 1, 2, 4, 24, 26, 35, 38])
C0 = (1.0 / (S * S + 81)) ** 0.5
C2 = (5.0 / (18 * S + 81)) ** 0.5
DD = (5.0 / (3 * H * H)) ** 0.5

TPAIRS = [(u, v) for u in range(9) for v in range(u + 1, 9)]     # 36
SPAIRS = [(u, v) for u in range(16) for v in range(u + 1, 16)]   # 120

# ---------------------------------------------------------------------------
# CP factors of C222 (symmetric). Fallback: baseline rank-13 + 5 identity dirs
# (nu=1 on the identity dirs gives the exact Gram path). If a fitted
# rank-R + tight-frame solution is provided, the identity dirs are dropped.
# ---------------------------------------------------------------------------
ASYM13 = np.array([[ 0.2047078304993985 ,-0.02548683359407013, 0.7272382102103669 ,-0.2704580317002371 , 0.09837678436495051, 0.33917102586453507, 0.0702064199526067 , 0.5084911526521594 ,-0.45926938484350616, 0.02051018350271685, 0.42935279562152645, 0.11369761887680929,-0.9795087183109351 ],
 [-0.44463451059315895,-0.1475020911181585 ,-0.08599458327748657,-0.3399741021461676 ,-0.22682371559002337, 0.28678061126448023, 0.7650776592713625 , 0.26958836857825846, 0.5278386781630274 , 0.3282383438246536 , 0.08456455835271014, 0.5900296552329473 , 0.02292460782275062],
 [-0.4191472648275923 ,-0.26988537144017594,-0.4746358369743323 ,-0.05420760101850775,-0.4844170977223217 ,-0.2282388774655017 ,-0.19634218768794168, 0.08251395533362854, 0.06550659513246503, 0.1133290752849004 , 0.303297071331556  ,-0.5334125260375588 ,-0.16883005035035203],
 [ 1.0431848707368094 ,-0.14228996865607693, 0.1431376570259985 , 0.8838003679813345 ,-0.11670899310031788, 0.21559606010496696, 0.04691847768104187, 0.7674518688427294 , 1.1874537614603238 ,-0.10792487711796182,-0.17088928262877545, 0.09873177011237796,-0.6460911914396512 ],
 [-0.1789979061960668 , 0.6605325263316313 ,-0.04352029718970135, 0.19469438466538228, 0.12156843143529865, 0.5275314988902706 ,-0.7974376738648722 , 0.01570195752313255,-0.1619243884486304 , 0.467054091034758  , 0.16454467309626772,-0.11642783633169705, 0.01683699581923372]])
LAM13 = np.array([-0.3368296096552994 , 1.2424482608763587 , 0.6885666883749189 , 0.5830944196804277 ,-2.0867522613313056 , 0.21906413438838154, 0.4065311860292724 , 0.7616168984284204 ,-0.16794491943022935,-1.819621132649064  ,-0.6515708567347953 ,-0.6783119354005673 , 0.34147667194459136])

# Optional fitted factors (rank R + tight frame); populated by fit if found.
FIT = None  # dict(A=(R,5), lam=(R,), nu=(R,)) or None


def _factors():
    if FIT is not None:
        A = np.asarray(FIT["A"], np.float64)
        lam = np.asarray(FIT["lam"], np.float64)
        nu = np.asarray(FIT["nu"], np.float64)
        R = A.shape[0]
        return R, A, A, lam, nu           # R, A(cg), D(dirs), lamh2, nuh0
    A = ASYM13.T.astype(np.float64)       # (13,5)
    lam = LAM13.astype(np.float64)
    D = np.concatenate([A, np.eye(5)], axis=0)     # (18,5)
    lamh2 = np.concatenate([lam, np.zeros(5)])
    nuh0 = np.concatenate([np.zeros(13), np.ones(5)])
    return 13, A, D, lamh2, nuh0


def _chunks(n, step=128):
    return [(lo, min(n, lo + step)) for lo in range(0, n, step)]


def _static_plan():
    R, A, D, lamh2, nuh0 = _factors()
    ND = D.shape[0]
    NSQ = ND * 45 + 136
    N6 = R * 32
    plan = {
        "R": R, "ND": ND, "NSQ": NSQ, "N6": N6,
        "sq_chunks": _chunks(NSQ), "f4_chunks": _chunks(720),
        "e_chunks": _chunks(N6),
    }
    # zero-block structure of the consume weights
    sq_a, sq_b = [], []
    for lo, hi in plan["sq_chunks"]:
        rows = np.arange(lo, hi)
        t_rows = rows[rows < ND * 45]
        has_a = bool(np.any(lamh2[t_rows // 45] != 0.0)) if len(t_rows) else False
        has_b = True   # every chunk feeds h0 (nu or s-part)
        sq_a.append(has_a); sq_b.append(has_b)
    plan["sq_a"], plan["sq_b"] = sq_a, sq_b
    plan["f4_a"] = [True, True, True, True, False]
    plan["f4_b"] = [False, False, False, False, True]
    return plan


PLAN = _static_plan()


def _const_shapes():
    p = PLAN
    shp = {}
    for c, (lo, hi) in enumerate(p["sq_chunks"]):
        n = hi - lo
        shp[f"PR{c}"] = (80, n)
        if p["sq_a"][c]:
            shp[f"CWa{c}"] = (n, 128)
        if p["sq_b"][c]:
            shp[f"CWb{c}"] = (n, 64)
    shp["FW"] = (80, 128)
    for k in range(5):
        shp[f"G4_{k}"] = (45, 128)
        shp[f"C4_{k}"] = (128, 128 if k < 5 - 1 else 64)
    shp["YWa"] = (64, 128)
    shp["YWb"] = (64, 32)
    shp["W5a"] = (128, 5)
    shp["W5b"] = (32, 5)
    for c, (lo, hi) in enumerate(p["e_chunks"]):
        n = hi - lo
        shp[f"EWa{c}"] = (128, n)
        shp[f"EWb{c}"] = (32, n)
        shp[f"W6{c}"] = (n, 5)
    return shp


CONST_SHAPES = _const_shapes()


def _pair_weights(ws):
    """ws: (n,n,32) symmetric. -> (n + n(n-1)/2, 32) so that
    sum_uv ws[uvw] x_u x_v = sum_q Wq[q,w] * square_q(x)."""
    n = ws.shape[0]
    pairs = [(u, v) for u in range(n) for v in range(u + 1, n)]
    Wq = np.zeros((n + len(pairs), ws.shape[2]))
    for u in range(n):
        Wq[u] = ws[u, u] - sum(ws[u, v] for v in range(n) if v != u)
    for q, (u, v) in enumerate(pairs):
        Wq[n + q] = ws[u, v]
    return Wq


R4 = 128


def _fit_w2(W2, iters=60):
    """Rank-R4 CP of W2 (16,9,32) via ALS; returns f(R,16), g(R,9), h(R,32)."""
    best = None
    nrm = np.linalg.norm(W2)
    Wu = W2.reshape(16, 9 * 32)
    Wv = W2.transpose(1, 0, 2).reshape(9, 16 * 32)
    Ww = W2.transpose(2, 0, 1).reshape(32, 16 * 9)
    for seed in (0, 1, 2):
        rng = np.random.default_rng(seed)
        g = rng.normal(size=(R4, 9)); h = rng.normal(size=(R4, 32))
        f = rng.normal(size=(R4, 16))
        for it in range(iters):
            M = np.einsum("rv,rw->rvw", g, h).reshape(R4, -1)
            f = np.linalg.lstsq(M.T, Wu.T, rcond=None)[0]
            M = np.einsum("ru,rw->ruw", f, h).reshape(R4, -1)
            g = np.linalg.lstsq(M.T, Wv.T, rcond=None)[0]
            M = np.einsum("ru,rv->ruv", f, g).reshape(R4, -1)
            h = np.linalg.lstsq(M.T, Ww.T, rcond=None)[0]
        err = np.linalg.norm(np.einsum("ru,rv,rw->uvw", f, g, h) - W2) / nrm
        if best is None or err < best[0]:
            best = (err, f.T.copy(), g.T.copy(), h.T.copy())
        if err < 1e-3:
            break
    err, fT, gT, hT = best
    if err > 5e-3:
        raise RuntimeError(f"W2 CP fit failed: rel err {err:.2e}")
    # balance norms: |f_r| = |g_r| = 1, h carries magnitude
    f, g, h = fT.T, gT.T, hT.T
    nf = np.linalg.norm(f, axis=1) + 1e-30
    ng = np.linalg.norm(g, axis=1) + 1e-30
    f = f / nf[:, None]; g = g / ng[:, None]; h = h * (nf * ng)[:, None]
    return f, g, h


def _build_consts(w000, w110, w011, w101, w111, v010, v100, v110, mm_dt=None):
    p = PLAN
    R, A, D, lamh2, nuh0 = _factors()
    ND = D.shape[0]
    NSQ = p["NSQ"]

    w000s = (w000 + w000.transpose(1, 0, 2)) / 2
    w110s = (w110 + w110.transpose(1, 0, 2)) / 2
    w111s = (w111 + w111.transpose(1, 0, 2)) / 2
    W2 = w011 + w101.transpose(1, 0, 2)
    vs = (v110 + v110.T) / 2
    mu, Eig = np.linalg.eigh(vs)
    Vp = v010 + v100.T

    Wt111 = _pair_weights(w111s)   # (45,32)
    Wt110 = _pair_weights(w110s)
    W0 = _pair_weights(w000s)      # (136,32)

    # PR (61, NSQ) static + CW (NSQ, 192) runtime
    PR = np.zeros((80, NSQ))
    CW = np.zeros((NSQ, 192))
    for r in range(ND):
        for q in range(45):
            row = 45 * r + q
            if q < 9:
                for i in range(5):
                    PR[5 * q + i, row] = D[r, i]
            else:
                u, v = TPAIRS[q - 9]
                for i in range(5):
                    PR[5 * u + i, row] += D[r, i]
                    PR[5 * v + i, row] += D[r, i]
            if lamh2[r] != 0.0:
                for k in range(5):
                    CW[row, 32 * k:32 * k + 32] = (C2 * lamh2[r] * D[r, k]) * Wt111[q]
            if nuh0[r] != 0.0:
                CW[row, 160:192] = (C0 * INV_SQRT5 * nuh0[r]) * Wt110[q]
    for q in range(136):
        row = ND * 45 + q
        if q < 16:
            PR[64 + q, row] = 1.0
        else:
            u, v = SPAIRS[q - 16]
            PR[64 + u, row] = 1.0
            PR[64 + v, row] = 1.0
        CW[row, 160:192] = C0 * W0[q]

    C = {}
    for c, (lo, hi) in enumerate(p["sq_chunks"]):
        C[f"PR{c}"] = PR[:, lo:hi]
        if p["sq_a"][c]:
            C[f"CWa{c}"] = CW[lo:hi, 0:128]
        if p["sq_b"][c]:
            C[f"CWb{c}"] = CW[lo:hi, 128:192]

    f4f, f4g, f4h = _fit_w2(W2)                    # (R4,16),(R4,9),(R4,32)
    FW = np.zeros((80, 128))
    FW[64:80, :] = f4f.T
    C["FW"] = FW
    for k in range(5):
        Gk = np.zeros((45, 128))
        for v in range(9):
            Gk[5 * v + k, :] = f4g[:, v]
        C[f"G4_{k}"] = Gk
        ncol = 128 if k < 4 else 64
        Ck = np.zeros((128, ncol))
        off = 32 * k if k < 4 else 0
        Ck[:, off:off + 32] = (C2 * INV_SQRT5) * f4h
        C[f"C4_{k}"] = Ck

    # stage 2
    YW = np.zeros((64, 160))
    W5 = np.zeros((160, 5))
    for k in range(5):
        for w in range(32):
            YW[32:64, 32 * k + w] = Vp[:, w]
            W5[32 * k + w, k] = DD * INV_SQRT5
    C["YWa"], C["YWb"] = YW[:, 0:128], YW[:, 128:160]
    C["W5a"], C["W5b"] = W5[0:128], W5[128:160]

    EW = np.zeros((160, p["N6"]))
    W6 = np.zeros((p["N6"], 5))
    for r in range(R):
        for a in range(32):
            col = 32 * r + a
            for k in range(5):
                EW[32 * k:32 * k + 32, col] = A[r, k] * Eig[:, a]
            W6[col] = DD * lamh2[r] * mu[a] * A[r]
    for c, (lo, hi) in enumerate(p["e_chunks"]):
        C[f"EWa{c}"] = EW[0:128, lo:hi]
        C[f"EWb{c}"] = EW[128:160, lo:hi]
        C[f"W6{c}"] = W6[lo:hi]

    if mm_dt is None:
        mm_dt = MM_DT
    out = {}
    for k, v in C.items():
        if mm_dt == "f32r" and _is_prep(k):
            out[k] = _tf32_round(v)
        else:
            out[k] = np.ascontiguousarray(v, dtype=_bf16np)
    return out


_NC_CACHE = {}

# tunables (sim-swept): psum bufs per tag, emission order
BUFS = {"sqp": 2, "tbp": 2, "o1a": 1, "o1b": 1, "fin": 2}
F4_FIRST = False


def build_nc(b_core=B_CORE, repeat=1, mm_dt=None):
    import concourse.bacc as bacc
    import concourse.mybir as mybir
    import concourse.tile as tile

    f32 = mybir.dt.float32
    bf16 = mybir.dt.bfloat16
    f32r = mybir.dt.float32r
    if mm_dt is None:
        mm_dt = MM_DT
    prep_dt = f32r if mm_dt == "f32r" else bf16
    p = PLAN
    nt = b_core // NB
    nc = bacc.Bacc()
    mm = nc.tensor.matmul

    xt_dram = nc.dram_tensor("xt_t", (80, b_core), prep_dt, kind="ExternalInput")
    cdram = {k: nc.dram_tensor(k, shp, prep_dt if _is_prep(k) else bf16,
                               kind="ExternalInput")
             for k, shp in CONST_SHAPES.items()}
    out_dram = nc.dram_tensor("out_t", (5, b_core), f32, kind="ExternalOutput")

    nsqc = len(p["sq_chunks"])
    nf4c = len(p["f4_chunks"])
    nec = len(p["e_chunks"])

    # first/last writers of the o1a / o1b accumulation groups
    a_writers = [("sq", c) for c in range(nsqc) if p["sq_a"][c]] + \
                [("f4", k) for k in range(5) if p["f4_a"][k]]
    b_writers = [("sq", c) for c in range(nsqc) if p["sq_b"][c]] + \
                [("f4", k) for k in range(5) if p["f4_b"][k]]

    with tile.TileContext(nc) as tc:
        with (
            tc.tile_pool(name="consts", bufs=1) as cp,
            tc.tile_pool(name="io", bufs=3) as io,
            tc.tile_pool(name="wk", bufs=2) as wk,
            tc.tile_pool(name="psum", bufs=1, space="PSUM") as ps,
        ):
            ct = {}
            for k, shp in CONST_SHAPES.items():
                ct[k] = cp.tile(list(shp), prep_dt if _is_prep(k) else bf16,
                                tag=k, name=f"c_{k}")
                nc.sync.dma_start(ct[k][:], cdram[k][:])

            pending = []
            for it in range(nt * repeat):
                col = NB * (it % nt)
                sl = slice(col, col + NB)

                xt = io.tile([80, NB], prep_dt, tag="xt")
                nc.sync.dma_start(xt[:], xt_dram[:, sl])

                o1a = ps.tile([128, NB], f32, tag="o1a", bufs=BUFS["o1a"])
                o1b = ps.tile([64, NB], f32, tag="o1b", bufs=BUFS["o1b"])

                # ---- SQ family: prep -> square -> consume (skew-1 pipeline)
                def sq_cons(c, sqs):
                    if p["sq_a"][c]:
                        mm(o1a[:], ct[f"CWa{c}"][:], sqs[:],
                                         start=(a_writers[0] == ("sq", c)),
                                         stop=(a_writers[-1] == ("sq", c)))
                    if p["sq_b"][c]:
                        mm(o1b[:], ct[f"CWb{c}"][:], sqs[:],
                                         start=(b_writers[0] == ("sq", c)),
                                         stop=(b_writers[-1] == ("sq", c)))

                sq_tiles = {}
                for c, (lo, hi) in enumerate(p["sq_chunks"]):
                    n = hi - lo
                    sqp = ps.tile([n, NB], f32, tag="sqp", bufs=BUFS["sqp"])
                    mm(sqp[:], ct[f"PR{c}"][:], xt[:],
                                     start=True, stop=True)
                    sqs = wk.tile([n, NB], bf16, tag="sqs", bufs=3)
                    nc.scalar.square(sqs[:], sqp[:])
                    sq_tiles[c] = sqs
                    if c >= 1:
                        sq_cons(c - 1, sq_tiles.pop(c - 1))
                sq_cons(nsqc - 1, sq_tiles.pop(nsqc - 1))

                # ---- F4 family (CP-fit R4=128): OP1 once, per-k op2/mult/consume
                def f4_cons(k, pp):
                    if k < 4:
                        mm(o1a[:], ct[f"C4_{k}"][:], pp[:],
                           start=(a_writers[0] == ("f4", k)),
                           stop=(a_writers[-1] == ("f4", k)))
                    else:
                        mm(o1b[:], ct[f"C4_{k}"][:], pp[:],
                           start=(b_writers[0] == ("f4", k)),
                           stop=(b_writers[-1] == ("f4", k)))

                OP1ps = ps.tile([128, NB], f32, tag="tbp", bufs=BUFS["tbp"])
                mm(OP1ps[:], ct["FW"][:], xt[:], start=True, stop=True)
                OP1s = wk.tile([128, NB], bf16, tag="SAs", bufs=2)
                nc.vector.tensor_copy(OP1s[:], OP1ps[:])
                f4_tiles = {}
                for k in range(5):
                    tbp = ps.tile([128, NB], f32, tag="tbp", bufs=BUFS["tbp"])
                    mm(tbp[:], ct[f"G4_{k}"][:], xt[0:45, :], start=True, stop=True)
                    pp = wk.tile([128, NB], bf16, tag="pp", bufs=3)
                    nc.vector.tensor_mul(pp[:], tbp[:], OP1s[:])
                    f4_tiles[k] = pp
                    if k >= 1:
                        f4_cons(k - 1, f4_tiles.pop(k - 1))
                f4_cons(4, f4_tiles.pop(4))

                # ---- evict stage-1 results; stage 2 runs one tile deferred ----
                h2a = wk.tile([128, NB], bf16, tag="h2a", bufs=2)
                nc.scalar.copy(h2a[:], o1a[:])
                o1bs = wk.tile([64, NB], bf16, tag="o1bs", bufs=2)
                nc.scalar.copy(o1bs[:], o1b[:])
                pending.append((sl, h2a, o1bs))
                if len(pending) > 1 or it == nt * repeat - 1:
                    flush = [pending.pop(0)]
                    if it == nt * repeat - 1:
                        flush.append(pending.pop(0))
                else:
                    flush = []

                for sl2, h2a2, o1bs2 in flush:
                    fin = ps.tile([5, NB], f32, tag="fin", bufs=BUFS["fin"])
                    yrA = ps.tile([128, NB], f32, tag="sqp", bufs=BUFS["sqp"])
                    mm(yrA[:], ct["YWa"][:], o1bs2[:], start=True, stop=True)
                    p5a = wk.tile([128, NB], bf16, tag="p5", bufs=2)
                    nc.vector.tensor_mul(p5a[:], yrA[:], h2a2[:])
                    mm(fin[:], ct["W5a"][:], p5a[:], start=True, stop=False)
                    yrB = ps.tile([32, NB], f32, tag="tbp", bufs=BUFS["tbp"])
                    mm(yrB[:], ct["YWb"][:], o1bs2[:], start=True, stop=True)
                    p5b = wk.tile([32, NB], bf16, tag="p5b", bufs=2)
                    nc.vector.tensor_mul(p5b[:], yrB[:], o1bs2[0:32, :])
                    mm(fin[:], ct["W5b"][:], p5b[:], start=False, stop=False)

                    for c, (lo, hi) in enumerate(p["e_chunks"]):
                        n = hi - lo
                        ep = ps.tile([n, NB], f32, tag="sqp", bufs=BUFS["sqp"])
                        mm(ep[:], ct[f"EWa{c}"][:, :], h2a2[:], start=True, stop=False)
                        mm(ep[:], ct[f"EWb{c}"][:, :], o1bs2[0:32, :], start=False, stop=True)
                        es = wk.tile([n, NB], bf16, tag="es", bufs=2)
                        nc.scalar.square(es[:], ep[:])
                        mm(fin[:], ct[f"W6{c}"][:], es[:], start=False, stop=(c == nec - 1))

                    outs = wk.tile([5, NB], f32, tag="outs", bufs=2)
                    nc.vector.tensor_copy(outs[:], fin[:])
                    nc.sync.dma_start(out_dram[:, sl2], outs[:])

    nc.compile()
    return nc


def _host_prep(scalars, kernel_t2s, mm_dt=None):
    s = np.asarray(scalars, np.float32)
    kt = np.asarray(kernel_t2s, np.float32)
    t45 = np.concatenate([kt[:, L2_IDX, :], kt.sum(axis=1, keepdims=True)],
                         axis=1).reshape(B_FULL, 45)
    xt = np.zeros((B_FULL, 80), np.float32)
    xt[:, 0:45] = t45
    xt[:, 64:80] = s
    if mm_dt is None:
        mm_dt = MM_DT
    xtt = np.ascontiguousarray(xt.T)
    if mm_dt == "f32r":
        return _tf32_round(xtt)
    return xtt.astype(_bf16np)


def kernel(scalars, kernel_t2s, w000, w110, w011, w101, w111, v010, v100, v110):
    from concourse.bass_utils import run_bass_kernel_spmd

    consts = _build_consts(
        np.asarray(w000, np.float64), np.asarray(w110, np.float64),
        np.asarray(w011, np.float64), np.asarray(w101, np.float64),
        np.asarray(w111, np.float64), np.asarray(v010, np.float64),
        np.asarray(v100, np.float64), np.asarray(v110, np.float64))
    xt = _host_prep(scalars, kernel_t2s)

    if "nc" not in _NC_CACHE:
        _NC_CACHE["nc"] = build_nc()
    nc = _NC_CACHE["nc"]

    in_maps = []
    for c in range(N_CORES):
        sl = slice(c * B_CORE, (c + 1) * B_CORE)
        m = {"xt_t": np.ascontiguousarray(xt[:, sl])}
        m.update(consts)
        in_maps.append(m)

    res = run_bass_kernel_spmd(nc, in_maps, core_ids=list(range(N_CORES)))
    out = np.empty((B_FULL, 5), np.float32)
    for c in range(N_CORES):
        out[c * B_CORE:(c + 1) * B_CORE] = res.results[c]["out_t"].T
    return out
